# revision 53
# baseline (speedup 1.0000x reference)
"""Gemma3 sliding-window attention kernel for 8 Trainium2 NeuronCores.

Sharding: core c handles batch b = c//4, query-row chunk j = c%4 (512 rows).
The reference keeps only the LAST 512 key columns for every query row, so
each core computes k/v projections for rows 1536:2048 of its batch — all 4
kv heads locally (no collectives; the duplicated kv compute is cheaper than
the AllGather latency on hardware).

All matmul operands stream from HBM in bf16 (cast host-side); PSUM
accumulation is fp32 and softmax math stays fp32.

The attention works in HEAD PAIRS (each q-head pair shares one kv head, so
scores/softmax operands and the tanh scale coincide): DVE/ACT chain ops run
on [128, 2*512] pair tiles, halving the per-op overhead that dominates on
hardware. RMS normalization is deferred off the critical path: khat/qhat
hold rope((1+w)*raw); rs_k folds into the softcap tanh's per-partition
scale AP; rs_q is applied to qhat one pipeline step later from a
PE-broadcast row of sums of squares, with rsqrt done as a cubic seed + two
Newton steps (bf16 then fp32). Pipeline: step s issues qproj(pair s),
scores(s-2), attn_out(s-3).
"""

import numpy as np
import ml_dtypes

import concourse.bacc as bacc
import concourse.tile as tile
from concourse import mybir
from concourse.bass_utils import run_bass_kernel_spmd

F32 = mybir.dt.float32
F32R = mybir.dt.float32r
BF16 = mybir.dt.bfloat16
AF = mybir.ActivationFunctionType
OP = mybir.AluOpType

B, L, HID = 2, 2048, 2560
NH, NKV, D = 8, 4, 256
NP = NH // 2       # head pairs; pair p = heads (2p, 2p+1), kv head p
W = 512            # effective kv window (last W positions of the sequence)
CH = 512           # query rows per core
NCORES = 8
KT = HID // 128    # 20 contraction tiles for the projections
EPS = 1e-6
SOFTCAP = 50.0
SCALE = D ** -0.5
ROPE_BASE = 10000.0
NPBF16 = ml_dtypes.bfloat16
# tanh input scale c folded into the rs_k rsqrt: tanh((c*rs_k) * (rs_q*s))
C0 = SCALE / SOFTCAP
# cubic minimax seed for rsqrt on t in [0.3, 3.2] (rel err 4.9%), then two
# Newton steps (first bf16, second fp32) -> ~1e-4
RSQ_P3, RSQ_P2, RSQ_P1, RSQ_P0 = (-0.11751866, 0.81282722,
                                  -1.93345784, 2.24612936)


def _build(loop_n=None):
    nc = bacc.Bacc("TRN2", target_bir_lowering=False, debug=False,
                   num_devices=NCORES)
    xq_d = nc.dram_tensor("xq", [128, KT, CH], BF16, kind="ExternalInput").ap()
    xkv_d = nc.dram_tensor("xkv", [128, KT, W], BF16, kind="ExternalInput").ap()
    qw_d = nc.dram_tensor("qw", [NP, 128, KT, 2 * D], BF16,
                          kind="ExternalInput").ap()
    kwh_d = nc.dram_tensor("kwh", [128, KT, 256], BF16, kind="ExternalInput").ap()
    vwh_d = nc.dram_tensor("vwh", [128, KT, 256], BF16, kind="ExternalInput").ap()
    ow_d = nc.dram_tensor("ow", [128, HID // 128, 16, 128], BF16,
                          kind="ExternalInput").ap()
    # rope tables duplicated over the pair dim for [128, 2, *] chain ops
    cq = nc.dram_tensor("cq", [128, 2, CH], BF16, kind="ExternalInput").ap()
    sq = nc.dram_tensor("sq", [128, 2, CH], BF16, kind="ExternalInput").ap()
    sqn = nc.dram_tensor("sqn", [128, 2, CH], BF16, kind="ExternalInput").ap()
    ck = nc.dram_tensor("ck", [128, 2, W], BF16, kind="ExternalInput").ap()
    sk = nc.dram_tensor("sk", [128, 2, W], BF16, kind="ExternalInput").ap()
    skn = nc.dram_tensor("skn", [128, 2, W], BF16, kind="ExternalInput").ap()
    # columns: 1+qnw[:128], 1+qnw[128:], 1+knw[:128], 1+knw[128:]
    w1p = nc.dram_tensor("w1p", [128, 4], F32, kind="ExternalInput").ap()
    # (1+w)^-2 correction columns so sums of squares of the (1+w)-scaled
    # copies recover the raw-q/k norms; same column order as w1p
    ccol_d = nc.dram_tensor("ccol", [128, 4], BF16, kind="ExternalInput").ap()
    onesc_d = nc.dram_tensor("onesc", [128, 1], BF16, kind="ExternalInput").ap()
    onesr_d = nc.dram_tensor("onesr", [1, 128], F32R, kind="ExternalInput").ap()
    ident_d = nc.dram_tensor("ident", [128, 128], F32R, kind="ExternalInput").ap()
    yT = nc.dram_tensor("yT", [HID, CH], F32, kind="ExternalOutput").ap()

    NKC = 4
    CKT = KT // NKC

    with tile.TileContext(nc) as tc, \
            nc.allow_low_precision(reason='bf16 matmul operands'):
        with (
            tc.tile_pool(name="const", bufs=1) as pc,
            tc.tile_pool(name="px", bufs=2) as px,
            tc.tile_pool(name="pkw", bufs=2) as pkw,
            tc.tile_pool(name="pow", bufs=3) as pow_,
            tc.tile_pool(name="pkv", bufs=1) as pkv,
            tc.tile_pool(name="pq", bufs=1) as pq,
            tc.tile_pool(name="ptmp", bufs=2) as ptmp,
            tc.tile_pool(name="prow", bufs=1) as prow,
            tc.tile_pool(name="pexp", bufs=2) as pexp,
            tc.tile_pool(name="pout", bufs=2) as pout,
            tc.tile_pool(name="pdram", bufs=1, space="DRAM") as pdram,
            tc.tile_pool(name="pp", bufs=3, space="PSUM") as pp,
        ):
            import contextlib
            loop_ctx = tc.For_i(0, loop_n, 1) if loop_n else contextlib.nullcontext()
            # constants
            ones_col = pc.tile([128, 1], BF16, tag="onesc")
            nc.scalar.dma_start(out=ones_col, in_=onesc_d)
            ones_row = pc.tile([1, 128], F32R, tag="onesr")
            nc.scalar.dma_start(out=ones_row, in_=onesr_d)
            ident_sb = pc.tile([128, 128], F32R, tag="ident")
            nc.scalar.dma_start(out=ident_sb, in_=ident_d)
            ck_sb = pc.tile([128, 2, W], BF16, tag="c1")
            sk_sb = pc.tile([128, 2, W], BF16, tag="c2")
            skn_sb = pc.tile([128, 2, W], BF16, tag="c3")
            cq_sb = pc.tile([128, 2, CH], BF16, tag="c1")
            sq_sb = pc.tile([128, 2, CH], BF16, tag="c2")
            sqn_sb = pc.tile([128, 2, CH], BF16, tag="c3")
            w1p_sb = pc.tile([128, 4], F32, tag="w1p")
            nc.scalar.dma_start(out=w1p_sb, in_=w1p)
            ccol_sb = pc.tile([128, 4], BF16, tag="ccol")
            nc.scalar.dma_start(out=ccol_sb, in_=ccol_d)
            rsk_sb = pc.tile([128, 4 * NKV], F32, tag="rsk")
            # warm the ACT table set (exp/tanh/square/copy) while the first
            # weight DMAs are still in flight
            warm = pc.tile([128, 1], BF16, tag="warm")
            nc.scalar.activation(warm, ones_col, AF.Tanh)

            def rsqrt_sb(out_sb, t_sb, nfree, scale=1.0):
                """out = scale * t^-0.5 for SBUF f32 t (t in ~[0.3, 3.2]):
                cubic Horner seed + Newton iter in bf16, then one fp32
                Newton iter. y^2 goes through ACT Square."""
                z = ptmp.tile([128, nfree], BF16, tag="nwA", bufs=1)
                nc.vector.tensor_scalar(z, t_sb, RSQ_P3, RSQ_P2,
                                        op0=OP.mult, op1=OP.add)
                z2 = ptmp.tile([128, nfree], BF16, tag="nwB", bufs=1)
                nc.vector.scalar_tensor_tensor(z2, z, 0.0, t_sb,
                                               op0=OP.add, op1=OP.mult)
                z3 = ptmp.tile([128, nfree], BF16, tag="nwA", bufs=1)
                nc.vector.scalar_tensor_tensor(z3, z2, RSQ_P1, t_sb,
                                               op0=OP.add, op1=OP.mult)
                y = ptmp.tile([128, nfree], BF16, tag="nwB", bufs=1)
                nc.vector.tensor_scalar(y, z3, 1.0, RSQ_P0,
                                        op0=OP.mult, op1=OP.add)
                for it in range(2):
                    dt_ = BF16 if it == 0 else F32
                    sqy = ptmp.tile([128, nfree], dt_, tag="nwA", bufs=1,
                                    name=f"sqy{it}")
                    nc.vector.tensor_mul(sqy, y, y)
                    u = ptmp.tile([128, nfree], dt_, tag="nwC", bufs=1,
                                  name=f"nwu{it}")
                    nc.vector.tensor_mul(u, sqy, t_sb)
                    v = ptmp.tile([128, nfree], dt_, tag="nwA", bufs=1,
                                  name=f"nwv{it}")
                    nc.vector.tensor_scalar(v, u, -0.5, 1.5,
                                            op0=OP.mult, op1=OP.add)
                    if it == 0:
                        y1 = ptmp.tile([128, nfree], BF16, tag="nwD", bufs=1)
                        nc.vector.tensor_mul(y1, y, v)
                        y = y1
                    else:
                        nc.vector.scalar_tensor_tensor(out_sb, v, scale, y,
                                                       op0=OP.mult,
                                                       op1=OP.mult)

            def psum_evac(ps0p, ps1p, wcol0, wcol1, nfree, nm, d1=2):
                """Evacuate the projection PSUM pair through ACT copies that
                fold in the (1+w) scale; bf16 outputs let the rope chain run
                at the 2x DVE rate and free the PSUM banks after two ops."""
                c0 = ptmp.tile([128, d1, nfree], BF16, tag="qc0", bufs=2,
                               name=f"c0{nm}")
                c1 = ptmp.tile([128, d1, nfree], BF16, tag="qc1", bufs=2,
                               name=f"c1{nm}")
                nc.scalar.activation(c0, ps0p, AF.Copy, scale=wcol0)
                nc.scalar.activation(c1, ps1p, AF.Copy, scale=wcol1)
                return c0, c1

            def wrope_pair(c0, c1, h0, h1, cos2, sin2, nsin2, nfree, d1=2):
                """rope from the bf16 copies into the h0 (first-half) and
                h1 (second-half) destination APs."""
                a = ptmp.tile([128, d1, nfree], BF16, tag="ra", bufs=1)
                b2 = ptmp.tile([128, d1, nfree], BF16, tag="rb", bufs=1)
                nc.vector.tensor_mul(a, c0, cos2)
                nc.vector.tensor_mul(b2, c0, sin2)
                bn = ptmp.tile([128, d1, nfree], BF16, tag="nwA", bufs=1)
                a2 = ptmp.tile([128, d1, nfree], BF16, tag="nwB", bufs=1)
                nc.vector.tensor_mul(bn, c1, nsin2)
                nc.vector.tensor_mul(a2, c1, cos2)
                nc.vector.tensor_add(h0, a, bn)
                nc.vector.tensor_add(h1, a2, b2)

            with loop_ctx:
                khat = pkv.tile([128, 2 * NKV, W], BF16, tag="khat")
                v_sb = pkv.tile([128, 4, NKV * D], BF16, tag="v")
                qhat = pq.tile([128, 2 * NH, CH], BF16, tag="qhat")

                # ---- Phase 1: OWN-head kv projection + 4-core AllGather ----
                # Core c projects only kv head c%4 (weights supplied per-core
                # in kwh/vwh); the batch group {4b..4b+3} AllGathers
                # khat / v / rs_k so each core skips 3/4 of the kv matmuls.
                xkv_sb = px.tile([128, KT, W], BF16, tag="x")
                # own-head k and v weights packed into one tile:
                # [:, :, 0:256] = k head, [:, :, 256:512] = v head
                kvw_sb = pkw.tile([128, KT, 512], BF16, tag="w",
                                  name="kvw")
                CHUNKS = [(0, 1), (1, 2), (2, 4), (4, 8), (8, 14), (14, 20)]
                for lo, hi in CHUNKS:
                    sl = slice(lo, hi)
                    nc.sync.dma_start(out=kvw_sb[:, sl, 0:256],
                                      in_=kwh_d[:, sl, :])
                    nc.sync.dma_start(out=xkv_sb[:, sl, :], in_=xkv_d[:, sl, :])
                nc.sync.dma_start(out=kvw_sb[:, :, 256:512], in_=vwh_d)
                nc.scalar.dma_start(out=ck_sb, in_=ck)
                nc.scalar.dma_start(out=sk_sb, in_=sk)
                nc.scalar.dma_start(out=skn_sb, in_=skn)

                khat_loc = pkv.tile([128, 2, W], BF16, tag="khat_loc")
                v_loc = pkv.tile([128, 4, 256], BF16, tag="v_loc")
                kps = pp.tile([128, 2, W], F32, tag="b2", name="kps")
                # accumulation groups must stay contiguous per PSUM region:
                # interleaving open start/stop groups that share a bank
                # corrupts the accumulation
                for m in range(2):
                    for kt in range(KT):
                        nc.tensor.matmul(
                            kps[:, m, :],
                            kvw_sb[:, kt, m * 128:(m + 1) * 128],
                            xkv_sb[:, kt, :],
                            start=(kt == 0), stop=(kt == KT - 1))
                kc = psum_evac(kps[:, 0:1, :], kps[:, 1:2, :],
                               w1p_sb[:, 2:3], w1p_sb[:, 3:4], W, "k", d1=1)
                sqk = [ptmp.tile([128, 1, W], BF16, tag="tA",
                                 name=f"sqk{m}") for m in range(2)]
                for m in range(2):
                    nc.scalar.activation(sqk[m], kc[m], AF.Square)
                ssTl = pp.tile([128, 4], F32, tag="b1", name="ssTl", bufs=2)
                for mlk in range(4):
                    for m in range(2):
                        nc.tensor.matmul(
                            ssTl[:, mlk:mlk + 1],
                            sqk[m][:, 0, mlk * 128:(mlk + 1) * 128],
                            ccol_sb[:, 2 + m:3 + m],
                            start=(m == 0), stop=(m == 1))
                wrope_pair(kc[0], kc[1], khat_loc[:, 0:1, :],
                           khat_loc[:, 1:2, :], ck_sb[:, 0:1, :],
                           sk_sb[:, 0:1, :], skn_sb[:, 0:1, :], W, d1=1)
                tk = ptmp.tile([128, 4], F32, tag="tq", bufs=1, name="tk")
                nc.vector.tensor_scalar(tk, ssTl, 1.0 / D, EPS,
                                        op0=OP.mult, op1=OP.add)
                rsk_loc = ptmp.tile([128, 4], F32, tag="rbC", bufs=1,
                                    name="rskloc")
                rsqrt_sb(rsk_loc, tk, 4, scale=C0)

                # v projection, own head only
                vps = pp.tile([128, 4, 256], F32, tag="b2", name="vps")
                for mm in range(4):
                    for kt in range(KT):
                        nc.tensor.matmul(
                            vps[:, mm, :],
                            xkv_sb[:, kt, mm * 128:(mm + 1) * 128],
                            kvw_sb[:, kt, 256:512],
                            start=(kt == 0), stop=(kt == KT - 1))
                nc.vector.tensor_copy(v_loc, vps)

                # AllGather within the 4-core batch group (single packed
                # bf16 buffer: khat 1024 | v 1024 | rsk-as-bf16 8)
                in_b = pdram.tile([128, 2056], BF16, tag="inb")
                out_b = pdram.tile([NKV, 128, 2056], BF16, tag="outb")
                nc.sync.dma_start(out=in_b[:, 0:1024],
                                  in_=khat_loc.rearrange("p a b -> p (a b)"))
                nc.sync.dma_start(out=in_b[:, 1024:2048],
                                  in_=v_loc.rearrange("p a b -> p (a b)"))
                nc.sync.dma_start(out=in_b[:, 2048:2056],
                                  in_=rsk_loc.bitcast(BF16))
                nc.gpsimd.collective_compute(
                    "AllGather", OP.bypass,
                    ins=[in_b[:]],
                    outs=[out_b[:]],
                    replica_groups=[[0, 1, 2, 3], [4, 5, 6, 7]])
                def kv_unpack():
                    # Issued from pipeline step 1 (not right after the
                    # collective): HWDGE queues are FIFO per engine, so
                    # issuing these collective-gated DMAs early would block
                    # every later qw/xq/ow DMA on the sync queue until the
                    # AllGather completes.
                    for j in range(NKV):
                        nc.sync.dma_start(
                            out=khat[:, 2 * j:2 * j + 2, :],
                            in_=out_b[j, :, 0:1024].rearrange(
                                "p (a b) -> p a b", a=2, b=W))
                        nc.sync.dma_start(
                            out=v_sb[:, :, j * 256:(j + 1) * 256],
                            in_=out_b[j, :, 1024:2048].rearrange(
                                "p (a b) -> p a b", a=4, b=256))
                        nc.sync.dma_start(
                            out=rsk_sb[:, 4 * j:4 * j + 4],
                            in_=out_b[j, :, 2048:2056].bitcast(F32))

                nc.sync.dma_start(out=cq_sb, in_=cq)
                nc.sync.dma_start(out=sq_sb, in_=sq)
                nc.sync.dma_start(out=sqn_sb, in_=sqn)

                # ---- Phase 2+3: pair pipeline  qproj(p) | scores(p-2) |
                #      attn_out(p-3) ----
                xq_sb = px.tile([128, KT, CH], BF16, tag="x")
                for c in range(NKC):
                    sl = slice(c * CKT, (c + 1) * CKT)
                    nc.sync.dma_start(out=xq_sb[:, sl, :], in_=xq_d[:, sl, :])
                aoT = px.tile([128, 2 * NH, CH], BF16, tag="x")

                qc_live = {}       # p -> [2 bf16 [128,2,CH] (1+w)-scaled copies]
                sqt_live = {}      # p -> [2 bf16 [128,2,CH] sq tiles]
                ssq_live = {}      # p -> transposed sums of squares [128,8] PSUM
                rsq8_live = {}     # p -> rs_q compact [128,8] f32
                rbq_live = {}      # p -> rs_q broadcast [128,2,CH] f32 PSUM
                exps_live = {}     # p -> exp tile [128, 4, 2, CH]
                dnrow_live = {}    # p -> [1, 2, CH] f32r
                qn_live = {}       # p -> normalized qhat [128, 4, CH] bf16
                qw_tiles = {}

                def qw_prefetch(p):
                    qw_t = pkw.tile([128, KT, 2 * D], BF16, tag="w",
                                    name=f"qwp{p}")
                    nc.sync.dma_start(out=qw_t, in_=qw_d[p])
                    qw_tiles[p] = qw_t

                def qproj_mms(p):
                    qw_t = qw_tiles.pop(p)
                    qps = [pp.tile([128, 2, CH], F32, tag="b2",
                                   name=f"qps{p}{m}") for m in range(2)]
                    for kt in range(KT):
                        for i in range(2):
                            for m in range(2):
                                nc.tensor.matmul(
                                    qps[m][:, i, :],
                                    qw_t[:, kt,
                                         i * 256 + m * 128:
                                         i * 256 + (m + 1) * 128],
                                    xq_sb[:, kt, :],
                                    start=(kt == 0), stop=(kt == KT - 1))
                    qc = psum_evac(qps[0], qps[1], w1p_sb[:, 0:1],
                                   w1p_sb[:, 1:2], CH, f"q{p}")
                    sqt = [ptmp.tile([128, 2, CH], BF16, tag="tA",
                                     name=f"sqt{p}{m}") for m in range(2)]
                    for m in range(2):
                        nc.scalar.activation(sqt[m], qc[m], AF.Square)
                    qc_live[p] = qc
                    sqt_live[p] = sqt

                def rope_chain(p):
                    qc = qc_live.pop(p)
                    wrope_pair(qc[0], qc[1],
                               qhat[:, 4 * p:4 * p + 4:2, :],
                               qhat[:, 4 * p + 1:4 * p + 4:2, :],
                               cq_sb, sq_sb, sqn_sb, CH)

                def ssq_mms(p):
                    """Transposed sums of squares: [128(row), 8] where
                    col idx = i*4 + ch covers (head-in-pair i, 128-row
                    chunk ch)."""
                    sqt = sqt_live.pop(p)
                    ssq = pp.tile([128, 8], F32, tag="b1", bufs=2,
                                  name=f"ssq{p}")
                    for i in range(2):
                        for ch in range(4):
                            idx = i * 4 + ch
                            for m in range(2):
                                nc.tensor.matmul(
                                    ssq[:, idx:idx + 1],
                                    sqt[m][:, i, ch * 128:(ch + 1) * 128],
                                    ccol_sb[:, m:m + 1],
                                    start=(m == 0), stop=(m == 1))
                    ssq_live[p] = ssq

                def newton_c(p):
                    """rs_q on the compact [128,8] layout (cheap on DVE)."""
                    ssq = ssq_live.pop(p)
                    t8 = ptmp.tile([128, 8], F32, tag="tq", bufs=1,
                                   name=f"t8{p}")
                    nc.vector.tensor_scalar(t8, ssq, 1.0 / D, EPS,
                                            op0=OP.mult, op1=OP.add)
                    rsq8 = ptmp.tile([128, 8], F32R, tag="rbB", bufs=2,
                                     name=f"rsq8{p}")
                    rsqrt_sb(rsq8, t8, 8)
                    rsq8_live[p] = rsq8

                def trans_bcast(p):
                    """Broadcast compact rs_q to [128, 2, CH] PSUM via
                    stride-0 lhsT x identity: out[d, i, ch*128+j] =
                    rsq8[j, i*4+ch] for every partition d."""
                    rsq8r = rsq8_live.pop(p)
                    rbq_ps = pp.tile([128, 2, CH], F32, tag="b2",
                                     name=f"rbq{p}")
                    for i in range(2):
                        for ch in range(4):
                            idx = i * 4 + ch
                            nc.tensor.matmul(
                                rbq_ps[:, i, ch * 128:(ch + 1) * 128],
                                rsq8r[:, idx:idx + 1].to_broadcast([128, 128]),
                                ident_sb,
                                start=True, stop=True)
                    rbq_live[p] = rbq_ps

                def qnorm_mul(p):
                    rbq = rbq_live.pop(p)
                    qn = pq.tile([128, 4, CH], BF16, tag="qhatn", bufs=2,
                                 name=f"qhatn{p}")
                    for i in range(2):
                        for dk in range(2):
                            nc.vector.tensor_mul(
                                qn[:, 2 * i + dk, :],
                                qhat[:, 4 * p + 2 * i + dk, :],
                                rbq[:, i, :])
                    qn_live[p] = qn

                def sps_softmax(p):
                    g = p
                    qn = qn_live.pop(p)
                    exps = pexp.tile([128, 4, 2, CH], BF16, tag="exps",
                                     name=f"exps{p}")
                    for mlk in range(4):
                        sps = pp.tile([128, 2, CH], F32, tag="b2",
                                      name=f"sps{p}{mlk}")
                        for i in range(2):
                            for dk in range(2):
                                nc.tensor.matmul(
                                    sps[:, i, :],
                                    khat[:, 2 * g + dk,
                                         mlk * 128:(mlk + 1) * 128],
                                    qn[:, 2 * i + dk, :],
                                    start=(dk == 0), stop=(dk == 1))
                        nc.scalar.activation(
                            sps, sps, AF.Tanh,
                            scale=rsk_sb[:, g * 4 + mlk:g * 4 + mlk + 1])
                        nc.scalar.activation(exps[:, mlk, :, :], sps, AF.Exp,
                                             scale=SOFTCAP)
                    exps_live[p] = exps

                def dn_part(p):
                    exps = exps_live[p]
                    dn_ps = pp.tile([1, 2, CH], F32, tag="b2",
                                    name=f"dn{p}")
                    for i in range(2):
                        for mlk in range(4):
                            nc.tensor.matmul(dn_ps[:, i, :], ones_col,
                                             exps[:, mlk, i, :],
                                             start=(mlk == 0),
                                             stop=(mlk == 3))
                    dnrow = prow.tile([1, 2, CH], F32R, tag="drow",
                                      name=f"dnrow{p}")
                    nc.scalar.copy(dnrow, dn_ps)
                    dnrow_live[p] = dnrow

                def ops_fin(p):
                    g = p
                    exps = exps_live.pop(p)
                    opst = []
                    for dh in range(2):
                        ops = pp.tile([128, 2, CH], F32, tag="b2",
                                      name=f"ops{p}{dh}")
                        for i in range(2):
                            for klk in range(4):
                                nc.tensor.matmul(
                                    ops[:, i, :],
                                    v_sb[:, klk,
                                         g * 256 + dh * 128:
                                         g * 256 + dh * 128 + 128],
                                    exps[:, klk, i, :],
                                    start=(klk == 0), stop=(klk == 3))
                        opst.append(ops)
                    rbat_ps = pp.tile([128, 2, CH], F32, tag="b2",
                                      name=f"rbat{p}")
                    dnrow = dnrow_live.pop(p)
                    for i in range(2):
                        nc.tensor.matmul(rbat_ps[:, i, :], ones_row,
                                         dnrow[:, i, :], start=True,
                                         stop=True)
                    rbat = ptmp.tile([128, 2, CH], F32, tag="rbC", bufs=1,
                                     name=f"rbat{p}")
                    nc.vector.reciprocal_approx_fast(rbat, rbat_ps)
                    for dh in range(2):
                        # aoT slots {4p+dh, 4p+2+dh}
                        nc.vector.tensor_mul(
                            aoT[:, 4 * p + dh:4 * p + dh + 3:2, :],
                            opst[dh], rbat)

                qw_prefetch(0)
                for s in range(NP + 3):
                    if 0 <= s - 3 < NP:
                        dn_part(s - 3)
                    if 0 <= s - 1 < NP:
                        newton_c(s - 1)
                    if 0 <= s - 3 < NP:
                        ops_fin(s - 3)
                    if 0 <= s - 1 < NP:
                        trans_bcast(s - 1)
                        qnorm_mul(s - 1)
                    if 0 <= s - 2 < NP:
                        sps_softmax(s - 2)
                    if s + 1 < NP:
                        qw_prefetch(s + 1)
                    if s == 1:
                        kv_unpack()
                    if s < NP:
                        qproj_mms(s)
                    if s < NP:
                        ssq_mms(s)
                    if s < NP:
                        rope_chain(s)

                # ---- Phase 4: o projection (outputs transposed: yT) ----
                for mp2 in range(HID // 256):
                    yps = pp.tile([128, 2, CH], F32, tag="b2",
                                  name=f"yps{mp2}")
                    for j in range(2):
                        mp = 2 * mp2 + j
                        owc = pow_.tile([128, 16, 128], BF16, tag="ow",
                                        name=f"ow{mp}")
                        nc.sync.dma_start(out=owc, in_=ow_d[:, mp, :, :])
                        for kk in range(16):
                            nc.tensor.matmul(yps[:, j, :], owc[:, kk, :],
                                             aoT[:, kk, :],
                                             start=(kk == 0), stop=(kk == 15))
                    yst = pout.tile([128, 2, CH], F32, tag="yst")
                    nc.scalar.copy(yst, yps)
                    for j in range(2):
                        nc.sync.dma_start(
                            out=yT[(2 * mp2 + j) * 128:
                                   (2 * mp2 + j + 1) * 128, :],
                            in_=yst[:, j, :])

    nc.compile()

    return nc


_NC_CACHE = {}


def _get_nc():
    if "nc" not in _NC_CACHE:
        _NC_CACHE["nc"] = _build()
    return _NC_CACHE["nc"]


def _rope_tables():
    inv_freq = 1.0 / (ROPE_BASE ** (np.arange(0, D, 2, dtype=np.float32) / D))
    t = np.arange(L, dtype=np.float32)
    freqs = np.outer(t, inv_freq)                     # (L, 128)
    return (np.ascontiguousarray(np.cos(freqs).T.astype(np.float32)),
            np.ascontiguousarray(np.sin(freqs).T.astype(np.float32)))


def _part_major(mat_t, free):
    """(HID_like, free) feature-major -> (128, KT_like, free) partition-major
    bf16 blocks: out[p, kt, f] = mat_t[kt*128 + p, f]."""
    r = mat_t.shape[0]
    return np.ascontiguousarray(
        mat_t.reshape(r // 128, 128, free).transpose(1, 0, 2).astype(NPBF16))


def _dup2(tab):
    """(128, N) f32 -> (128, 2, N) bf16 duplicated over dim 1."""
    return np.ascontiguousarray(
        np.repeat(tab[:, None, :], 2, axis=1).astype(NPBF16))


def _prep_in_maps(x, q_w, k_w, v_w, o_w, q_norm_w, k_norm_w):
    # q_w per pair: (128, KT, 512); feats = head_in_pair*256 + d
    qw_p = np.ascontiguousarray(
        q_w.reshape(NP, 2 * D, KT, 128).transpose(0, 3, 2, 1).astype(NPBF16))
    kwT = np.ascontiguousarray(k_w.T)                 # (HID, 1024)
    vwT = np.ascontiguousarray(v_w.T)
    # per-kv-head weight shards: core c projects head c % NKV
    kw_h = [_part_major(kwT[:, j * 256:(j + 1) * 256], 256)
            for j in range(NKV)]
    vw_h = [_part_major(vwT[:, j * 256:(j + 1) * 256], 256)
            for j in range(NKV)]
    # o_w: (128, 20, 16, 128); ow_p[p, mp, kk, f] = o_w[mp*128+f, kk*128+p]
    ow_p = np.ascontiguousarray(
        o_w.reshape(HID // 128, 128, 16, 128).transpose(3, 0, 2, 1)
        .astype(NPBF16))
    cosT, sinT = _rope_tables()                        # (128, L) each
    w1p = np.empty((128, 4), dtype=np.float32)
    w1p[:, 0] = 1.0 + q_norm_w[:128]
    w1p[:, 1] = 1.0 + q_norm_w[128:]
    w1p[:, 2] = 1.0 + k_norm_w[:128]
    w1p[:, 3] = 1.0 + k_norm_w[128:]
    ccol = (w1p ** -2).astype(NPBF16)

    kv_lo = L - W
    xkv_b = [_part_major(np.ascontiguousarray(x[b, kv_lo:, :].T), W)
             for b in range(B)]
    ck_t = _dup2(cosT[:, kv_lo:])
    sk_t = _dup2(sinT[:, kv_lo:])
    skn_t = _dup2(-sinT[:, kv_lo:])

    in_maps = []
    for c in range(NCORES):
        b, j = divmod(c, 4)
        rows = slice(j * CH, (j + 1) * CH)
        in_maps.append({
            "xq": _part_major(np.ascontiguousarray(x[b, rows, :].T), CH),
            "xkv": xkv_b[b],
            "qw": qw_p, "kwh": kw_h[c % NKV], "vwh": vw_h[c % NKV],
            "ow": ow_p,
            "cq": _dup2(cosT[:, rows]),
            "sq": _dup2(sinT[:, rows]),
            "sqn": _dup2(-sinT[:, rows]),
            "ck": ck_t, "sk": sk_t, "skn": skn_t,
            "w1p": w1p, "ccol": ccol,
            "onesc": np.ones((128, 1), dtype=NPBF16),
            "onesr": np.ones((1, 128), dtype=np.float32),
            "ident": np.eye(128, dtype=np.float32),
        })
    return in_maps


def kernel(x, mask, q_w, k_w, v_w, o_w, q_norm_w, k_norm_w):
    x = np.asarray(x, dtype=np.float32)
    q_w = np.asarray(q_w, dtype=np.float32)
    k_w = np.asarray(k_w, dtype=np.float32)
    v_w = np.asarray(v_w, dtype=np.float32)
    o_w = np.asarray(o_w, dtype=np.float32)
    q_norm_w = np.asarray(q_norm_w, dtype=np.float32)
    k_norm_w = np.asarray(k_norm_w, dtype=np.float32)

    nc = _get_nc()
    in_maps = _prep_in_maps(x, q_w, k_w, v_w, o_w, q_norm_w, k_norm_w)

    res = run_bass_kernel_spmd(nc, in_maps, list(range(NCORES)))
    _NC_CACHE["last_res"] = res

    out = np.empty((B, L, HID), dtype=np.float32)
    for c in range(NCORES):
        b, j = divmod(c, 4)
        out[b, j * CH:(j + 1) * CH, :] = res.results[c]["yT"].T
    return out

